# revision 17
# baseline (speedup 1.0000x reference)
"""Sparse transposed-conv (27-tap) + BN + LeakyReLU on 8 TRN2 cores — v2.

Scatter-free design (the baseline's dma_scatter_add CCE-RMW was ~1.7ms):
  Phase A (per round r of ~9 taps, per src window w): one batched dma_gather of
    x rows (fp32, 256B/row) for all round-taps; PE-transpose to channel-major;
    bf16 GEMM per tap against block-diag [[Wk,0],[0,Wk]] (2 rows per column,
    pair-packed so each column is tap-pure); PE-transpose back; DENSE write to
    an HBM contribution buffer C_r (bf16 rows padded to 128ch = 256B).
  Phase B (per round): dma_gather C_r rows in dst-major slot order (shared
    slot layout: each 128-dst block q owns max-over-cores slots, mult of 64);
    per 128-slot chunk one/two matmuls with the gathered chunk as stationary
    and a host-built 0/1 segment matrix streamed as rhs; accumulates h^T
    [64ch x 128dst] blocks in PSUM (memset + has_written semantics), added
    into an SBUF-resident hT [128, 12800] (even dst-chunk -> partitions 0:64,
    odd -> 64:128).
  Center tap: dense, feeds hT directly (no back-transpose).
  BN tail: free-dim reduces on hT, parity fold, 8-core AllReduce, fused
  Lrelu(h*s+b) on ACT, PE transpose back, y writes.
"""
import os
import numpy as np
import ml_dtypes

import concourse.bass as bass
import concourse.mybir as mybir
import concourse.bacc as bacc
import concourse.tile as tile
from concourse import bass_utils
from concourse.masks import make_identity

N = 200000
C = 8
V = N // C            # 25000 dst voxels per core
VP = 25600            # padded dst count (200 chunks of 128)
D = 64
K = 27
KC = 13               # center tap
WIN = 32768
NW = (N + WIN - 1) // WIN
NR = 3                # tap rounds
EPS = 1e-5
NEG = 0.01
F32 = mybir.dt.float32
BF16 = mybir.dt.bfloat16
I16 = mybir.dt.int16
NQ = 1                # SWDGE queues (multi-queue gave no overlap on HW)
BF = ml_dtypes.bfloat16
K2 = int(os.environ.get("K2PHASE", "4")) if os.environ.get("KERNEL_DEBUG") else 4


def _pack16(slab, col0, vals):
    w = vals.reshape(-1, 16).T
    L16 = w.shape[1]
    for r in range(8):
        slab[r * 16:(r + 1) * 16, col0:col0 + L16] = w
    return col0 + L16


def _prep_host(nbr):
    nbr = np.asarray(nbr, np.int64)
    taps = [k for k in range(K) if k != KC]
    rounds = [taps[0:9], taps[9:18], taps[18:26]]

    # per (c,k,w): (src_rel int32, dst_local int32) sorted by dst
    lists = {}
    for c in range(C):
        for k in taps:
            src = nbr[k, c * V:(c + 1) * V]
            valid = np.nonzero(src >= 0)[0]
            s = src[valid]
            for w in range(NW):
                m = (s >= w * WIN) & (s < (w + 1) * WIN)
                lists[(c, k, w)] = (s[m] - w * WIN, valid[m])

    # ---------- Phase A plan ----------
    # per (r,w): taps get column ranges (shared); positions = 2*cols
    aplan = []           # [r][w] -> dict(pos0, npos, tapcols=[(k,c0,c1)], ga0)
    CR = []              # C rows per round
    ga = 0
    for r, rtaps in enumerate(rounds):
        wplans = []
        pos0 = 0
        for w in range(NW):
            tapcols = []
            c0 = 0
            for k in rtaps:
                ncol = max((len(lists[(c, k, w)][0]) + 1) // 2 for c in range(C))
                tapcols.append((k, c0, c0 + ncol))
                c0 += ncol
            ncols = (c0 + 127) & ~127
            npos = 2 * ncols
            wplans.append(dict(pos0=pos0, npos=npos, tapcols=tapcols, ga0=ga))
            pos0 += npos
            ga += npos
        assert pos0 < 32768, f"round {r} C rows {pos0} exceed int16"
        aplan.append(wplans)
        CR.append(pos0)
    GA = ga

    # A idx slab + C positions of every contribution
    gaslab = np.zeros((C, 128, GA // 16), np.int16)
    cpos = {}            # (c,k,w) -> np.array of C positions (per round space)
    for c in range(C):
        idxA = np.zeros(GA, np.int16)
        for r in range(NR):
            for w in range(NW):
                wp = aplan[r][w]
                for (k, k0, k1) in wp["tapcols"]:
                    srcs, dsts = lists[(c, k, w)]
                    n = len(srcs)
                    j = np.arange(n)
                    col = k0 + j // 2
                    pos = 256 * (col // 128) + (col % 128) + 128 * (j % 2)
                    idxA[wp["ga0"] + pos] = srcs.astype(np.int16)
                    cpos[(c, k, w)] = wp["pos0"] + pos
        _pack16(gaslab[c], 0, idxA)

    # ---------- Phase B plan ----------
    # per round: per-core contributions (dst, cpos); shared slot layout per
    # 128-dst block q with L_q multiple of 64.
    NQB = VP // 128      # 200 dst chunks
    bplan = []           # [r] -> dict(gb0, nslots, chunks=[(s0,s1,q) subblocks])
    gb0 = 0
    percore_bc = {}      # (c, r) -> (dst array, cpos array) sorted by dst
    for r, rtaps in enumerate(rounds):
        nq = np.zeros((C, NQB), np.int64)
        for c in range(C):
            ds, ps = [], []
            for w in range(NW):
                for k in rtaps:
                    srcs, dsts = lists[(c, k, w)]
                    ds.append(dsts)
                    ps.append(cpos[(c, k, w)])
            ds = np.concatenate(ds)
            ps = np.concatenate(ps)
            o = np.argsort(ds, kind="stable")
            ds, ps = ds[o], ps[o]
            percore_bc[(c, r)] = (ds, ps)
            np.add.at(nq[c], ds // 128, 1)
        # L_q multiple of 128 so every 128-slot chunk maps to exactly one q
        # (partition-offset stationaries fault the PE — full-128 MMs only)
        Lq = ((nq.max(axis=0) + 127) // 128) * 128
        Sq = np.zeros(NQB + 1, np.int64)
        Sq[1:] = np.cumsum(Lq)
        nslots = int(Sq[-1])
        # chunk ci (128 slots) -> its dst block q
        qidx = np.searchsorted(Sq, np.arange(0, nslots, 128), side="right") - 1
        bplan.append(dict(gb0=gb0, nslots=nslots, chunks=qidx.tolist(), Sq=Sq,
                          Lq=Lq))
        gb0 += nslots
    GB = gb0

    gbslab = np.zeros((C, 128, GB // 16), np.int16)
    segslab = np.zeros((C, 128, GB), np.uint16)   # bf16 bit patterns
    ONE = np.float32(1.0).astype(BF).view(np.uint16)
    for c in range(C):
        idxB = np.zeros(GB, np.int16)
        for r in range(NR):
            bp = bplan[r]
            ds, ps = percore_bc[(c, r)]
            Sq = bp["Sq"]
            # slot index per contribution: Sq[q] + rank within q
            qv = ds // 128
            # ranks: ds sorted so within q contributions are consecutive
            starts = np.searchsorted(qv, np.arange(NQB))
            rank = np.arange(len(ds)) - starts[qv]
            slot = Sq[qv] + rank
            idxB[bp["gb0"] + slot] = ps.astype(np.int16)
            # seg: partition = slot%128, col = (slot//128)*128 + (dst%128)
            p = slot % 128
            col = bp["gb0"] + (slot // 128) * 128 + (ds % 128)
            segslab[c][p, col] = ONE
        _pack16(gbslab[c], 0, idxB)

    return dict(aplan=aplan, bplan=bplan, CR=CR, GA=GA, GB=GB), \
        gaslab, gbslab, segslab.view(BF)


# ----------------------------------------------------------------------------
_CURRENT = None


def _build_body(nc):
    plan = _CURRENT
    aplan, bplan, CR = plan["aplan"], plan["bplan"], plan["CR"]
    GA, GB = plan["GA"], plan["GB"]

    x_d = nc.dram_tensor("x_d", [N, D], F32, kind="ExternalInput")
    xc_d = nc.dram_tensor("xc_d", [VP, D], BF16, kind="ExternalInput")
    W_d = nc.dram_tensor("W_d", [K, D, D], BF16, kind="ExternalInput")
    gam_d = nc.dram_tensor("gam_d", [1, D], F32, kind="ExternalInput")
    bet_d = nc.dram_tensor("bet_d", [1, D], F32, kind="ExternalInput")
    ga_d = nc.dram_tensor("ga_d", [128, GA // 16], I16, kind="ExternalInput")
    gb_d = nc.dram_tensor("gb_d", [128, GB // 16], I16, kind="ExternalInput")
    seg_d = nc.dram_tensor("seg_d", [128, GB], BF16, kind="ExternalInput")
    y_d = nc.dram_tensor("y_d", [VP, D], F32, kind="ExternalOutput")

    with tile.TileContext(nc) as tc:
        with tc.tile_pool(name="sb", bufs=1) as sb, \
             tc.tile_pool(name="io", bufs=3) as io, \
             tc.tile_pool(name="ps", bufs=2, space="PSUM") as ps, \
             tc.tile_pool(name="dram", bufs=1, space="DRAM") as dram:

            c_bufs = [dram.tile([CR[r] + 256, 128], BF16, name=f"cbuf{r}")
                      for r in range(NR)]

            idf = sb.tile([128, 128], F32)
            make_identity(nc, idf[:])
            idb = sb.tile([128, 128], BF16)
            make_identity(nc, idb[:])

            ga_t = sb.tile([128, GA // 16], I16)
            nc.sync.dma_start(ga_t[:], ga_d[:, :])
            gb_t = sb.tile([128, GB // 16], I16)
            nc.sync.dma_start(gb_t[:], gb_d[:, :])

            W2 = sb.tile([128, K * 128], BF16)
            nc.gpsimd.memset(W2[:], 0.0)
            for k in range(K):
                nc.sync.dma_start(W2[0:D, k * 128:k * 128 + D], W_d[k, :, :])
                nc.sync.dma_start(W2[D:128, k * 128 + D:(k + 1) * 128], W_d[k, :, :])

            zt = sb.tile([128, 512], F32)
            nc.gpsimd.memset(zt[:], 0.0)

            hT = sb.tile([128, VP // 2], F32)   # [128, 12800]

            # ---- center tap: xc (m p) c view, transpose, MM, store to hT ----
            for g in range(25):
                xg = io.tile([128, 512], BF16, tag="xg")
                nc.sync.dma_start(
                    xg[:].rearrange("p (m c) -> p m c", m=8),
                    xc_d[1024 * g:1024 * (g + 1), :].rearrange(
                        "(m p) c -> p m c", m=8, p=128))
                pa = ps.tile([128, 512], BF16, tag="psAb", space="PSUM", bufs=1)
                for j in range(4):
                    nc.tensor.transpose(out=pa[:, j * 128:(j + 1) * 128],
                                        in_=xg[:, j * 128:(j + 1) * 128],
                                        identity=idb[:])
                ct = io.tile([128, 512], BF16, tag="ct")
                nc.vector.tensor_copy(ct[:], pa[:])
                pc = ps.tile([128, 512], F32, tag="psC", space="PSUM")
                nc.tensor.matmul(out=pc[:], lhsT=W2[:, KC * 128:(KC + 1) * 128],
                                 rhs=ct[:], start=True, stop=True)
                nc.vector.tensor_copy(hT[:, g * 512:(g + 1) * 512], pc[:])

            # ---- phase A: per (round, window) batched gather -> C_r ----
            for r in range(NR if K2 >= 2 else 0):
                for w in range(NW):
                    wp = aplan[r][w]
                    npos = wp["npos"]
                    if npos == 0:
                        continue
                    nslot = npos // 128
                    assert nslot <= 48, nslot
                    gbuf = io.tile([128, 48, D], F32, tag="gbuf")
                    nc.gpsimd.dma_gather(
                        out_ap=gbuf[:, 0:nslot, :],
                        in_ap=x_d[w * WIN:min(N, (w + 1) * WIN), :],
                        idxs_ap=ga_t[:, wp["ga0"] // 16:(wp["ga0"] + npos) // 16],
                        num_idxs=npos, num_idxs_reg=npos, elem_size=D,
                        single_packet=False, queue_num=(r * NW + w) % NQ)
                    gbf = gbuf[:, 0:nslot, :].rearrange("p m d -> p (m d)")
                    ncols = npos // 2
                    for g0 in range(0, ncols, 512):
                        gw = min(512, ncols - g0)
                        pa = ps.tile([128, 512], F32, tag="psA", space="PSUM")
                        for j in range(gw // 128):
                            nc.tensor.transpose(
                                out=pa[:, j * 128:(j + 1) * 128],
                                in_=gbf[:, g0 + j * 128:g0 + (j + 1) * 128],
                                identity=idf[:])
                        ct = io.tile([128, 512], BF16, tag="ct")
                        nc.scalar.activation(ct[:, 0:gw], pa[:, 0:gw],
                                             mybir.ActivationFunctionType.Copy,
                                             bias=0.0)
                        pb = ps.tile([128, 512], F32, tag="psB", space="PSUM")
                        for (k, k0, k1) in wp["tapcols"]:
                            a, b = max(k0, g0), min(k1, g0 + gw)
                            if b <= a:
                                continue
                            nc.tensor.matmul(
                                out=pb[:, a - g0:b - g0],
                                lhsT=W2[:, k * 128:(k + 1) * 128],
                                rhs=ct[:, a - g0:b - g0], start=True, stop=True)
                        hb = io.tile([128, 512], BF16, tag="hb")
                        nc.vector.tensor_copy(hb[:, 0:gw], pb[:, 0:gw])
                        pc = ps.tile([128, 512], BF16, tag="psCb", space="PSUM", bufs=1)
                        for j in range(gw // 128):
                            nc.tensor.transpose(
                                out=pc[:, j * 128:(j + 1) * 128],
                                in_=hb[:, j * 128:(j + 1) * 128],
                                identity=idb[:])
                        so = io.tile([128, 512], BF16, tag="so")
                        nc.scalar.activation(so[:, 0:gw], pc[:, 0:gw],
                                             mybir.ActivationFunctionType.Copy,
                                             bias=0.0)
                        p0 = wp["pos0"] + 2 * g0
                        nc.sync.dma_start(
                            c_bufs[r][p0:p0 + 2 * gw, 0:64].rearrange(
                                "(m p) c -> p m c", p=128),
                            so[:, 0:gw].rearrange("p (m c) -> p m c", c=64))

            sacc = sb.tile([128, 32], F32)
            qacc = sb.tile([128, 32], F32)
            nc.gpsimd.memset(sacc[:], 0.0)
            nc.gpsimd.memset(qacc[:], 0.0)
            # ---- phase B: per round dst-major gather + seg matmuls ----
            NG = VP // 1024    # 25 psum groups of 1024 dsts
            for r in range(NR if K2 >= 3 else 0):
                bp = bplan[r]
                nslots = bp["nslots"]
                nchunk = nslots // 128
                chunks = bp["chunks"]
                # sub-gathers of up to 32 chunks
                SUB = 32
                subs = [(s, min(s + SUB, nchunk)) for s in range(0, nchunk, SUB)]
                gtiles = {}
                for si, (c0, c1) in enumerate(subs):
                    g2 = io.tile([128, SUB, 128], BF16, tag="g2")
                    nc.gpsimd.dma_gather(
                        out_ap=g2[:, 0:c1 - c0, :],
                        in_ap=c_bufs[r][:, :],
                        idxs_ap=gb_t[:, (bp["gb0"] + c0 * 128) // 16:
                                     (bp["gb0"] + c1 * 128) // 16],
                        num_idxs=(c1 - c0) * 128, num_idxs_reg=(c1 - c0) * 128,
                        elem_size=128, single_packet=False)
                    g2f = g2[:].rearrange("p m d -> p (m d)")
                    for ci in range(c0, c1):
                        gtiles[ci] = (g2f, ci - c0)
                # seg stream tiles of 16 chunks (2048 cols)
                segtiles = {}
                for s0 in range(0, nchunk, 16):
                    s1 = min(s0 + 16, nchunk)
                    st = io.tile([128, 16 * 128], BF16, tag="st")
                    nc.sync.dma_start(
                        st[:, 0:(s1 - s0) * 128],
                        seg_d[:, bp["gb0"] + s0 * 128:bp["gb0"] + s1 * 128])
                    for ci in range(s0, s1):
                        segtiles[ci] = (st, ci - s0)
                # psum groups
                for t in range(NG):
                    pe = ps.tile([128, 512], F32, tag="psA", space="PSUM")
                    po = ps.tile([128, 512], F32, tag="psB", space="PSUM")
                    nc.vector.tensor_copy(pe[0:64, :], zt[0:64, :])
                    nc.vector.tensor_copy(po[0:64, :], zt[0:64, :])
                    qlo, qhi = 8 * t, 8 * t + 8
                    for ci in range(nchunk):
                        q = chunks[ci]
                        if q < qlo or q >= qhi:
                            continue
                        g2f, m = gtiles[ci]
                        st, sc = segtiles[ci]
                        dst = pe if q % 2 == 0 else po
                        cb = ((q - qlo) // 2) * 128
                        nc.tensor.matmul(
                            out=dst[0:64, cb:cb + 128],
                            lhsT=g2f[0:128, m * 128:m * 128 + 64],
                            rhs=st[0:128, sc * 128:(sc + 1) * 128],
                            start=False, stop=True)
                    cols = slice(512 * t, 512 * (t + 1))
                    nc.vector.tensor_tensor(out=hT[0:64, cols], in0=pe[0:64, :],
                                            in1=hT[0:64, cols],
                                            op=mybir.AluOpType.add)
                    nc.vector.tensor_tensor(out=hT[64:128, cols], in0=po[0:64, :],
                                            in1=hT[64:128, cols],
                                            op=mybir.AluOpType.add)
                    if r == NR - 1:
                        ch = hT[:, 512 * t:512 * (t + 1)]
                        nc.vector.tensor_reduce(out=sacc[:, t:t + 1], in_=ch,
                                                axis=mybir.AxisListType.X,
                                                op=mybir.AluOpType.add)
                        scr = io.tile([128, 512], F32, tag="scr")
                        nc.vector.tensor_tensor(out=scr[:], in0=ch, in1=ch,
                                                op=mybir.AluOpType.mult)
                        nc.vector.tensor_reduce(out=qacc[:, t:t + 1], in_=scr[:],
                                                axis=mybir.AxisListType.X,
                                                op=mybir.AluOpType.add)

            # ---- BN stats folded into last round's group loop ----
            sq = sb.tile([128, 2], F32)
            nc.vector.tensor_reduce(out=sq[:, 0:1], in_=sacc[:, 0:25],
                                    axis=mybir.AxisListType.X,
                                    op=mybir.AluOpType.add)
            nc.vector.tensor_reduce(out=sq[:, 1:2], in_=qacc[:, 0:25],
                                    axis=mybir.AxisListType.X,
                                    op=mybir.AluOpType.add)
            sqh = sb.tile([64, 2], F32)
            nc.sync.dma_start(sqh[:], sq[64:128, :])
            sq64 = sb.tile([64, 2], F32)
            nc.vector.tensor_tensor(out=sq64[:], in0=sq[0:64, :], in1=sqh[:],
                                    op=mybir.AluOpType.add)
            cc_in = dram.tile([64, 2], F32)
            cc_out = dram.tile([64, 2], F32)
            nc.gpsimd.dma_start(cc_in[:], sq64[:])
            nc.gpsimd.collective_compute(
                "AllReduce", mybir.AluOpType.add,
                replica_groups=[list(range(C))],
                ins=[cc_in.opt()], outs=[cc_out.opt()])
            g2r = sb.tile([64, 2], F32)
            nc.sync.dma_start(g2r[:], cc_out[:])
            me = sb.tile([64, 2], F32)
            nc.vector.tensor_scalar_mul(me[:], g2r[:], 1.0 / N)
            v1 = sb.tile([64, 1], F32)
            nc.vector.tensor_tensor(out=v1[:], in0=me[:, 0:1], in1=me[:, 0:1],
                                    op=mybir.AluOpType.mult)
            nc.vector.tensor_tensor(out=v1[:], in0=me[:, 1:2], in1=v1[:],
                                    op=mybir.AluOpType.subtract)
            eps_t = sb.tile([64, 1], F32)
            nc.gpsimd.memset(eps_t[:], EPS)
            std = sb.tile([64, 1], F32)
            nc.scalar.activation(std[:], v1[:], mybir.ActivationFunctionType.Sqrt,
                                 bias=eps_t[:])
            rin = sb.tile([64, 1], F32)
            nc.vector.reciprocal(rin[:], std[:])
            gam = sb.tile([64, 1], F32)
            nc.sync.dma_start(gam[:], gam_d[0, :, None])
            bet = sb.tile([64, 1], F32)
            nc.sync.dma_start(bet[:], bet_d[0, :, None])
            sc_h = sb.tile([64, 1], F32)
            nc.vector.tensor_tensor(out=sc_h[:], in0=rin[:], in1=gam[:],
                                    op=mybir.AluOpType.mult)
            cb_h = sb.tile([64, 1], F32)
            nc.vector.tensor_tensor(out=cb_h[:], in0=me[:, 0:1], in1=sc_h[:],
                                    op=mybir.AluOpType.mult)
            nc.vector.tensor_tensor(out=cb_h[:], in0=bet[:], in1=cb_h[:],
                                    op=mybir.AluOpType.subtract)
            s128 = sb.tile([128, 2], F32)
            nc.sync.dma_start(s128[0:64, 0:1], sc_h[:])
            nc.sync.dma_start(s128[64:128, 0:1], sc_h[:])
            nc.sync.dma_start(s128[0:64, 1:2], cb_h[:])
            nc.sync.dma_start(s128[64:128, 1:2], cb_h[:])

            # ---- apply + transpose back + write y ----
            for g in range(25):
                ha = io.tile([128, 512], F32, tag="ha")
                nc.scalar.activation(ha[:], hT[:, g * 512:(g + 1) * 512],
                                     mybir.ActivationFunctionType.Lrelu,
                                     bias=s128[:, 1:2], scale=s128[:, 0:1],
                                     alpha=NEG)
                pc = ps.tile([128, 512], F32, tag="psC", space="PSUM")
                for j in range(4):
                    nc.tensor.transpose(out=pc[:, j * 128:(j + 1) * 128],
                                        in_=ha[:, j * 128:(j + 1) * 128],
                                        identity=idf[:])
                yo = io.tile([128, 512], F32, tag="yo")
                nc.vector.tensor_copy(yo[:], pc[:])
                nc.sync.dma_start(
                    y_d[1024 * g:1024 * (g + 1), :].rearrange(
                        "(a p) c -> p a c", a=8, p=128),
                    yo[:].rearrange("p (a c) -> p a c", c=64))


def _build_program(plan):
    global _CURRENT
    _CURRENT = plan
    nc = bacc.Bacc("TRN2", target_bir_lowering=False, debug=False,
                   num_devices=C, num_swdge_queues=NQ)
    _build_body(nc)
    nc.compile()
    return nc


_CACHE = {}


def build(nbr):
    key = nbr.tobytes()[:4096] + nbr.tobytes()[-4096:]
    if key in _CACHE:
        return _CACHE[key]
    plan, gaslab, gbslab, segslab = _prep_host(np.asarray(nbr))
    nc = _build_program(plan)
    _CACHE[key] = (nc, gaslab, gbslab, segslab)
    return _CACHE[key]


def kernel(x, W, gamma, beta, nbr):
    x = np.ascontiguousarray(np.asarray(x, np.float32))
    Wb = np.asarray(W, np.float32).astype(BF)
    gamma = np.asarray(gamma, np.float32).reshape(1, D)
    beta = np.asarray(beta, np.float32).reshape(1, D)
    nbr = np.asarray(nbr)
    nc, gaslab, gbslab, segslab = build(nbr)
    in_maps = []
    for c in range(C):
        xc = np.zeros((VP, D), BF)
        xc[:V] = x[c * V:(c + 1) * V].astype(BF)
        in_maps.append({
            "x_d": x,
            "xc_d": xc,
            "W_d": Wb,
            "gam_d": gamma,
            "bet_d": beta,
            "ga_d": gaslab[c],
            "gb_d": gbslab[c],
            "seg_d": segslab[c],
        })
    res = bass_utils.run_bass_kernel_spmd(nc, in_maps, core_ids=list(range(C)))
    return np.concatenate([res.results[c]["y_d"][:V] for c in range(C)], axis=0)


# revision 18
# speedup vs baseline: 1.7555x; 1.7555x over previous
"""Sparse transposed-conv (27-tap) + BN + LeakyReLU on 8 TRN2 cores — v2.

Scatter-free design (the baseline's dma_scatter_add CCE-RMW was ~1.7ms):
  Phase A (per round r of ~9 taps, per src window w): one batched dma_gather of
    x rows (fp32, 256B/row) for all round-taps; PE-transpose to channel-major;
    bf16 GEMM per tap against block-diag [[Wk,0],[0,Wk]] (2 rows per column,
    pair-packed so each column is tap-pure); PE-transpose back; DENSE write to
    an HBM contribution buffer C_r (bf16 rows padded to 128ch = 256B).
  Phase B (per round): dma_gather C_r rows in dst-major slot order (shared
    slot layout: each 128-dst block q owns max-over-cores slots, mult of 64);
    per 128-slot chunk one/two matmuls with the gathered chunk as stationary
    and a host-built 0/1 segment matrix streamed as rhs; accumulates h^T
    [64ch x 128dst] blocks in PSUM (memset + has_written semantics), added
    into an SBUF-resident hT [128, 12800] (even dst-chunk -> partitions 0:64,
    odd -> 64:128).
  Center tap: dense, feeds hT directly (no back-transpose).
  BN tail: free-dim reduces on hT, parity fold, 8-core AllReduce, fused
  Lrelu(h*s+b) on ACT, PE transpose back, y writes.
"""
import os
import numpy as np
import ml_dtypes

import concourse.bass as bass
import concourse.mybir as mybir
import concourse.bacc as bacc
import concourse.tile as tile
from concourse import bass_utils
from concourse.masks import make_identity

N = 200000
C = 8
V = N // C            # 25000 dst voxels per core
VP = 25600            # padded dst count (200 chunks of 128)
D = 64
K = 27
KC = 13               # center tap
WIN = 32768
NW = (N + WIN - 1) // WIN
NR = 3                # tap rounds
EPS = 1e-5
NEG = 0.01
F32 = mybir.dt.float32
BF16 = mybir.dt.bfloat16
I16 = mybir.dt.int16
NQ = 1                # SWDGE queues (multi-queue gave no overlap on HW)
BF = ml_dtypes.bfloat16
K2 = int(os.environ.get("K2PHASE", "4")) if os.environ.get("KERNEL_DEBUG") else 4


def _pack16(slab, col0, vals):
    w = vals.reshape(-1, 16).T
    L16 = w.shape[1]
    for r in range(8):
        slab[r * 16:(r + 1) * 16, col0:col0 + L16] = w
    return col0 + L16


def _prep_host(nbr):
    nbr = np.asarray(nbr, np.int64)
    taps = [k for k in range(K) if k != KC]
    rounds = [taps[0:9], taps[9:18], taps[18:26]]

    # per (c,k,w): (src_rel int32, dst_local int32) sorted by dst
    lists = {}
    for c in range(C):
        for k in taps:
            src = nbr[k, c * V:(c + 1) * V]
            valid = np.nonzero(src >= 0)[0]
            s = src[valid]
            for w in range(NW):
                m = (s >= w * WIN) & (s < (w + 1) * WIN)
                lists[(c, k, w)] = (s[m] - w * WIN, valid[m])

    # ---------- Phase A plan ----------
    # per (r,w): taps get column ranges (shared); positions = 2*cols
    aplan = []           # [r][w] -> dict(pos0, npos, tapcols=[(k,c0,c1)], ga0)
    CR = []              # C rows per round
    ga = 0
    for r, rtaps in enumerate(rounds):
        wplans = []
        pos0 = 0
        for w in range(NW):
            tapcols = []
            c0 = 0
            for k in rtaps:
                ncol = max((len(lists[(c, k, w)][0]) + 1) // 2 for c in range(C))
                tapcols.append((k, c0, c0 + ncol))
                c0 += ncol
            ncols = (c0 + 127) & ~127
            npos = 2 * ncols
            wplans.append(dict(pos0=pos0, npos=npos, tapcols=tapcols, ga0=ga))
            pos0 += npos
            ga += npos
        assert pos0 < 32768, f"round {r} C rows {pos0} exceed int16"
        aplan.append(wplans)
        CR.append(pos0)
    GA = ga

    # A idx slab + C positions of every contribution
    gaslab = np.zeros((C, 128, GA // 16), np.int16)
    cpos = {}            # (c,k,w) -> np.array of C positions (per round space)
    for c in range(C):
        idxA = np.zeros(GA, np.int16)
        for r in range(NR):
            for w in range(NW):
                wp = aplan[r][w]
                for (k, k0, k1) in wp["tapcols"]:
                    srcs, dsts = lists[(c, k, w)]
                    n = len(srcs)
                    j = np.arange(n)
                    col = k0 + j // 2
                    pos = 256 * (col // 128) + (col % 128) + 128 * (j % 2)
                    idxA[wp["ga0"] + pos] = srcs.astype(np.int16)
                    cpos[(c, k, w)] = wp["pos0"] + pos
        _pack16(gaslab[c], 0, idxA)

    # ---------- Phase B plan ----------
    # per round: per-core contributions (dst, cpos); shared slot layout per
    # 128-dst block q with L_q multiple of 64.
    NQB = VP // 128      # 200 dst chunks
    bplan = []           # [r] -> dict(gb0, nslots, chunks=[(s0,s1,q) subblocks])
    gb0 = 0
    percore_bc = {}      # (c, r) -> (dst array, cpos array) sorted by dst
    for r, rtaps in enumerate(rounds):
        nq = np.zeros((C, NQB), np.int64)
        for c in range(C):
            ds, ps = [], []
            for w in range(NW):
                for k in rtaps:
                    srcs, dsts = lists[(c, k, w)]
                    ds.append(dsts)
                    ps.append(cpos[(c, k, w)])
            ds = np.concatenate(ds)
            ps = np.concatenate(ps)
            o = np.argsort(ds, kind="stable")
            ds, ps = ds[o], ps[o]
            percore_bc[(c, r)] = (ds, ps)
            np.add.at(nq[c], ds // 128, 1)
        # L_q multiple of 128 so every 128-slot chunk maps to exactly one q
        # (partition-offset stationaries fault the PE — full-128 MMs only)
        Lq = ((nq.max(axis=0) + 127) // 128) * 128
        Sq = np.zeros(NQB + 1, np.int64)
        Sq[1:] = np.cumsum(Lq)
        nslots = int(Sq[-1])
        # chunk ci (128 slots) -> its dst block q
        qidx = np.searchsorted(Sq, np.arange(0, nslots, 128), side="right") - 1
        bplan.append(dict(gb0=gb0, nslots=nslots, chunks=qidx.tolist(), Sq=Sq,
                          Lq=Lq))
        gb0 += nslots
    GB = gb0

    gbslab = np.zeros((C, 128, GB // 16), np.int16)
    segslab = np.zeros((C, 128, GB), np.uint16)   # bf16 bit patterns
    ONE = np.float32(1.0).astype(BF).view(np.uint16)
    for c in range(C):
        idxB = np.zeros(GB, np.int16)
        for r in range(NR):
            bp = bplan[r]
            ds, ps = percore_bc[(c, r)]
            Sq = bp["Sq"]
            # slot index per contribution: Sq[q] + rank within q
            qv = ds // 128
            # ranks: ds sorted so within q contributions are consecutive
            starts = np.searchsorted(qv, np.arange(NQB))
            rank = np.arange(len(ds)) - starts[qv]
            slot = Sq[qv] + rank
            idxB[bp["gb0"] + slot] = ps.astype(np.int16)
            # seg: partition = slot%128, col = (slot//128)*128 + (dst%128)
            p = slot % 128
            col = bp["gb0"] + (slot // 128) * 128 + (ds % 128)
            segslab[c][p, col] = ONE
        _pack16(gbslab[c], 0, idxB)

    return dict(aplan=aplan, bplan=bplan, CR=CR, GA=GA, GB=GB), \
        gaslab, gbslab, segslab.view(BF)


# ----------------------------------------------------------------------------
_CURRENT = None


def _build_body(nc):
    plan = _CURRENT
    aplan, bplan, CR = plan["aplan"], plan["bplan"], plan["CR"]
    GA, GB = plan["GA"], plan["GB"]

    x_d = nc.dram_tensor("x_d", [N, D], F32, kind="ExternalInput")
    xc_d = nc.dram_tensor("xc_d", [VP, D], BF16, kind="ExternalInput")
    W_d = nc.dram_tensor("W_d", [K, D, D], BF16, kind="ExternalInput")
    gam_d = nc.dram_tensor("gam_d", [1, D], F32, kind="ExternalInput")
    bet_d = nc.dram_tensor("bet_d", [1, D], F32, kind="ExternalInput")
    ga_d = nc.dram_tensor("ga_d", [128, GA // 16], I16, kind="ExternalInput")
    gb_d = nc.dram_tensor("gb_d", [128, GB // 16], I16, kind="ExternalInput")
    seg_d = nc.dram_tensor("seg_d", [128, GB], BF16, kind="ExternalInput")
    y_d = nc.dram_tensor("y_d", [VP, D], F32, kind="ExternalOutput")

    with tile.TileContext(nc) as tc:
        with tc.tile_pool(name="sb", bufs=1) as sb, \
             tc.tile_pool(name="io", bufs=3) as io, \
             tc.tile_pool(name="ps", bufs=2, space="PSUM") as ps, \
             tc.tile_pool(name="dram", bufs=1, space="DRAM") as dram:

            c_bufs = [dram.tile([CR[r] + 256, 128], BF16, name=f"cbuf{r}")
                      for r in range(NR)]

            idf = sb.tile([128, 128], F32)
            make_identity(nc, idf[:])
            idb = sb.tile([128, 128], BF16)
            make_identity(nc, idb[:])

            ga_t = sb.tile([128, GA // 16], I16)
            nc.sync.dma_start(ga_t[:], ga_d[:, :])
            gb_t = sb.tile([128, GB // 16], I16)
            nc.sync.dma_start(gb_t[:], gb_d[:, :])

            W2 = sb.tile([128, K * 128], BF16)
            nc.gpsimd.memset(W2[:], 0.0)
            for k in range(K):
                nc.sync.dma_start(W2[0:D, k * 128:k * 128 + D], W_d[k, :, :])
                nc.sync.dma_start(W2[D:128, k * 128 + D:(k + 1) * 128], W_d[k, :, :])

            zt = sb.tile([128, 512], F32)
            nc.gpsimd.memset(zt[:], 0.0)

            hT = sb.tile([128, VP // 2], F32)   # [128, 12800]

            # ---- center tap: xc (m p) c view, transpose, MM, store to hT ----
            for g in range(25):
                xg = io.tile([128, 512], BF16, tag="xg")
                nc.sync.dma_start(
                    xg[:].rearrange("p (m c) -> p m c", m=8),
                    xc_d[1024 * g:1024 * (g + 1), :].rearrange(
                        "(m p) c -> p m c", m=8, p=128))
                pa = ps.tile([128, 512], BF16, tag="psAb", space="PSUM", bufs=1)
                for j in range(4):
                    nc.tensor.transpose(out=pa[:, j * 128:(j + 1) * 128],
                                        in_=xg[:, j * 128:(j + 1) * 128],
                                        identity=idb[:])
                ct = io.tile([128, 512], BF16, tag="ct")
                nc.vector.tensor_copy(ct[:], pa[:])
                pc = ps.tile([128, 512], F32, tag="psC", space="PSUM")
                nc.tensor.matmul(out=pc[:], lhsT=W2[:, KC * 128:(KC + 1) * 128],
                                 rhs=ct[:], start=True, stop=True)
                nc.vector.tensor_copy(hT[:, g * 512:(g + 1) * 512], pc[:])

            # ---- phase A: per (round, window) batched gather -> C_r ----
            for r in range(NR if K2 >= 2 else 0):
                for w in range(NW):
                    wp = aplan[r][w]
                    npos = wp["npos"]
                    if npos == 0:
                        continue
                    nslot = npos // 128
                    assert nslot <= 48, nslot
                    gbuf = io.tile([128, 48, D], F32, tag="gbuf")
                    nc.gpsimd.dma_gather(
                        out_ap=gbuf[:, 0:nslot, :],
                        in_ap=x_d[w * WIN:min(N, (w + 1) * WIN), :],
                        idxs_ap=ga_t[:, wp["ga0"] // 16:(wp["ga0"] + npos) // 16],
                        num_idxs=npos, num_idxs_reg=npos, elem_size=D,
                        single_packet=False, queue_num=(r * NW + w) % NQ)
                    gbf = gbuf[:, 0:nslot, :].rearrange("p m d -> p (m d)")
                    ncols = npos // 2
                    for g0 in range(0, ncols, 512):
                        gw = min(512, ncols - g0)
                        pa = ps.tile([128, 512], F32, tag="psA", space="PSUM")
                        for j in range(gw // 128):
                            nc.tensor.transpose(
                                out=pa[:, j * 128:(j + 1) * 128],
                                in_=gbf[:, g0 + j * 128:g0 + (j + 1) * 128],
                                identity=idf[:])
                        ct = io.tile([128, 512], BF16, tag="ct")
                        nc.vector.tensor_copy(ct[:, 0:gw], pa[:, 0:gw])
                        pb = ps.tile([128, 512], F32, tag="psB", space="PSUM")
                        for (k, k0, k1) in wp["tapcols"]:
                            a, b = max(k0, g0), min(k1, g0 + gw)
                            if b <= a:
                                continue
                            nc.tensor.matmul(
                                out=pb[:, a - g0:b - g0],
                                lhsT=W2[:, k * 128:(k + 1) * 128],
                                rhs=ct[:, a - g0:b - g0], start=True, stop=True)
                        hb = io.tile([128, 512], BF16, tag="hb")
                        nc.vector.tensor_copy(hb[:, 0:gw], pb[:, 0:gw])
                        pc = ps.tile([128, 512], BF16, tag="psCb", space="PSUM", bufs=1)
                        for j in range(gw // 128):
                            nc.tensor.transpose(
                                out=pc[:, j * 128:(j + 1) * 128],
                                in_=hb[:, j * 128:(j + 1) * 128],
                                identity=idb[:])
                        so = io.tile([128, 512], BF16, tag="so")
                        nc.scalar.activation(so[:, 0:gw], pc[:, 0:gw],
                                             mybir.ActivationFunctionType.Copy,
                                             bias=0.0)
                        p0 = wp["pos0"] + 2 * g0
                        nc.sync.dma_start(
                            c_bufs[r][p0:p0 + 2 * gw, 0:64].rearrange(
                                "(m p) c -> p m c", p=128),
                            so[:, 0:gw].rearrange("p (m c) -> p m c", c=64))

            # ---- phase B: per round dst-major gather + seg matmuls ----
            NG = VP // 1024    # 25 psum groups of 1024 dsts
            for r in range(NR if K2 >= 3 else 0):
                bp = bplan[r]
                nslots = bp["nslots"]
                nchunk = nslots // 128
                chunks = bp["chunks"]
                # sub-gathers of up to 32 chunks
                SUB = 32
                subs = [(s, min(s + SUB, nchunk)) for s in range(0, nchunk, SUB)]
                gtiles = {}
                for si, (c0, c1) in enumerate(subs):
                    g2 = io.tile([128, SUB, 128], BF16, tag="g2")
                    nc.gpsimd.dma_gather(
                        out_ap=g2[:, 0:c1 - c0, :],
                        in_ap=c_bufs[r][:, :],
                        idxs_ap=gb_t[:, (bp["gb0"] + c0 * 128) // 16:
                                     (bp["gb0"] + c1 * 128) // 16],
                        num_idxs=(c1 - c0) * 128, num_idxs_reg=(c1 - c0) * 128,
                        elem_size=128, single_packet=False)
                    g2f = g2[:].rearrange("p m d -> p (m d)")
                    for ci in range(c0, c1):
                        gtiles[ci] = (g2f, ci - c0)
                # seg stream tiles of 16 chunks (2048 cols)
                segtiles = {}
                for s0 in range(0, nchunk, 16):
                    s1 = min(s0 + 16, nchunk)
                    st = io.tile([128, 16 * 128], BF16, tag="st")
                    nc.sync.dma_start(
                        st[:, 0:(s1 - s0) * 128],
                        seg_d[:, bp["gb0"] + s0 * 128:bp["gb0"] + s1 * 128])
                    for ci in range(s0, s1):
                        segtiles[ci] = (st, ci - s0)
                # psum groups
                for t in range(NG):
                    pe = ps.tile([128, 512], F32, tag="psA", space="PSUM")
                    po = ps.tile([128, 512], F32, tag="psB", space="PSUM")
                    nc.vector.tensor_copy(pe[0:64, :], zt[0:64, :])
                    nc.vector.tensor_copy(po[0:64, :], zt[0:64, :])
                    qlo, qhi = 8 * t, 8 * t + 8
                    for ci in range(nchunk):
                        q = chunks[ci]
                        if q < qlo or q >= qhi:
                            continue
                        g2f, m = gtiles[ci]
                        st, sc = segtiles[ci]
                        dst = pe if q % 2 == 0 else po
                        cb = ((q - qlo) // 2) * 128
                        nc.tensor.matmul(
                            out=dst[0:64, cb:cb + 128],
                            lhsT=g2f[0:128, m * 128:m * 128 + 64],
                            rhs=st[0:128, sc * 128:(sc + 1) * 128],
                            start=False, stop=True)
                    cols = slice(512 * t, 512 * (t + 1))
                    nc.vector.tensor_tensor(out=hT[0:64, cols], in0=pe[0:64, :],
                                            in1=hT[0:64, cols],
                                            op=mybir.AluOpType.add)
                    nc.vector.tensor_tensor(out=hT[64:128, cols], in0=po[0:64, :],
                                            in1=hT[64:128, cols],
                                            op=mybir.AluOpType.add)

            # ---- BN stats on hT ----
            sacc = sb.tile([128, 32], F32)
            qacc = sb.tile([128, 32], F32)
            nc.gpsimd.memset(sacc[:], 0.0)
            nc.gpsimd.memset(qacc[:], 0.0)
            for g in range(25):
                ch = hT[:, g * 512:(g + 1) * 512]
                nc.vector.tensor_reduce(out=sacc[:, g:g + 1], in_=ch,
                                        axis=mybir.AxisListType.X,
                                        op=mybir.AluOpType.add)
                scr = io.tile([128, 512], F32, tag="scr")
                nc.vector.tensor_tensor(out=scr[:], in0=ch, in1=ch,
                                        op=mybir.AluOpType.mult)
                nc.vector.tensor_reduce(out=qacc[:, g:g + 1], in_=scr[:],
                                        axis=mybir.AxisListType.X,
                                        op=mybir.AluOpType.add)
            sq = sb.tile([128, 2], F32)
            nc.vector.tensor_reduce(out=sq[:, 0:1], in_=sacc[:, 0:25],
                                    axis=mybir.AxisListType.X,
                                    op=mybir.AluOpType.add)
            nc.vector.tensor_reduce(out=sq[:, 1:2], in_=qacc[:, 0:25],
                                    axis=mybir.AxisListType.X,
                                    op=mybir.AluOpType.add)
            sqh = sb.tile([64, 2], F32)
            nc.sync.dma_start(sqh[:], sq[64:128, :])
            sq64 = sb.tile([64, 2], F32)
            nc.vector.tensor_tensor(out=sq64[:], in0=sq[0:64, :], in1=sqh[:],
                                    op=mybir.AluOpType.add)
            cc_in = dram.tile([64, 2], F32)
            cc_out = dram.tile([64, 2], F32)
            nc.gpsimd.dma_start(cc_in[:], sq64[:])
            nc.gpsimd.collective_compute(
                "AllReduce", mybir.AluOpType.add,
                replica_groups=[list(range(C))],
                ins=[cc_in.opt()], outs=[cc_out.opt()])
            g2r = sb.tile([64, 2], F32)
            nc.sync.dma_start(g2r[:], cc_out[:])
            me = sb.tile([64, 2], F32)
            nc.vector.tensor_scalar_mul(me[:], g2r[:], 1.0 / N)
            v1 = sb.tile([64, 1], F32)
            nc.vector.tensor_tensor(out=v1[:], in0=me[:, 0:1], in1=me[:, 0:1],
                                    op=mybir.AluOpType.mult)
            nc.vector.tensor_tensor(out=v1[:], in0=me[:, 1:2], in1=v1[:],
                                    op=mybir.AluOpType.subtract)
            eps_t = sb.tile([64, 1], F32)
            nc.gpsimd.memset(eps_t[:], EPS)
            std = sb.tile([64, 1], F32)
            nc.scalar.activation(std[:], v1[:], mybir.ActivationFunctionType.Sqrt,
                                 bias=eps_t[:])
            rin = sb.tile([64, 1], F32)
            nc.vector.reciprocal(rin[:], std[:])
            gam = sb.tile([64, 1], F32)
            nc.sync.dma_start(gam[:], gam_d[0, :, None])
            bet = sb.tile([64, 1], F32)
            nc.sync.dma_start(bet[:], bet_d[0, :, None])
            sc_h = sb.tile([64, 1], F32)
            nc.vector.tensor_tensor(out=sc_h[:], in0=rin[:], in1=gam[:],
                                    op=mybir.AluOpType.mult)
            cb_h = sb.tile([64, 1], F32)
            nc.vector.tensor_tensor(out=cb_h[:], in0=me[:, 0:1], in1=sc_h[:],
                                    op=mybir.AluOpType.mult)
            nc.vector.tensor_tensor(out=cb_h[:], in0=bet[:], in1=cb_h[:],
                                    op=mybir.AluOpType.subtract)
            s128 = sb.tile([128, 2], F32)
            nc.sync.dma_start(s128[0:64, 0:1], sc_h[:])
            nc.sync.dma_start(s128[64:128, 0:1], sc_h[:])
            nc.sync.dma_start(s128[0:64, 1:2], cb_h[:])
            nc.sync.dma_start(s128[64:128, 1:2], cb_h[:])

            # ---- apply + transpose back + write y ----
            for g in range(25):
                ha = io.tile([128, 512], F32, tag="ha")
                nc.scalar.activation(ha[:], hT[:, g * 512:(g + 1) * 512],
                                     mybir.ActivationFunctionType.Lrelu,
                                     bias=s128[:, 1:2], scale=s128[:, 0:1],
                                     alpha=NEG)
                pc = ps.tile([128, 512], F32, tag="psC", space="PSUM")
                for j in range(4):
                    nc.tensor.transpose(out=pc[:, j * 128:(j + 1) * 128],
                                        in_=ha[:, j * 128:(j + 1) * 128],
                                        identity=idf[:])
                yo = io.tile([128, 512], F32, tag="yo")
                nc.vector.tensor_copy(yo[:], pc[:])
                nc.sync.dma_start(
                    y_d[1024 * g:1024 * (g + 1), :].rearrange(
                        "(a p) c -> p a c", a=8, p=128),
                    yo[:].rearrange("p (a c) -> p a c", c=64))


def _build_program(plan):
    global _CURRENT
    _CURRENT = plan
    nc = bacc.Bacc("TRN2", target_bir_lowering=False, debug=False,
                   num_devices=C, num_swdge_queues=NQ)
    _build_body(nc)
    nc.compile()
    return nc


_CACHE = {}


def build(nbr):
    key = nbr.tobytes()[:4096] + nbr.tobytes()[-4096:]
    if key in _CACHE:
        return _CACHE[key]
    plan, gaslab, gbslab, segslab = _prep_host(np.asarray(nbr))
    nc = _build_program(plan)
    _CACHE[key] = (nc, gaslab, gbslab, segslab)
    return _CACHE[key]


def kernel(x, W, gamma, beta, nbr):
    x = np.ascontiguousarray(np.asarray(x, np.float32))
    Wb = np.asarray(W, np.float32).astype(BF)
    gamma = np.asarray(gamma, np.float32).reshape(1, D)
    beta = np.asarray(beta, np.float32).reshape(1, D)
    nbr = np.asarray(nbr)
    nc, gaslab, gbslab, segslab = build(nbr)
    in_maps = []
    for c in range(C):
        xc = np.zeros((VP, D), BF)
        xc[:V] = x[c * V:(c + 1) * V].astype(BF)
        in_maps.append({
            "x_d": x,
            "xc_d": xc,
            "W_d": Wb,
            "gam_d": gamma,
            "bet_d": beta,
            "ga_d": gaslab[c],
            "gb_d": gbslab[c],
            "seg_d": segslab[c],
        })
    res = bass_utils.run_bass_kernel_spmd(nc, in_maps, core_ids=list(range(C)))
    return np.concatenate([res.results[c]["y_d"][:V] for c in range(C)], axis=0)


# revision 19
# speedup vs baseline: 2.4820x; 1.4138x over previous
"""Sparse transposed-conv (27-tap) + BN + LeakyReLU on 8 TRN2 cores — v2.

Scatter-free design (the baseline's dma_scatter_add CCE-RMW was ~1.7ms):
  Phase A (per round r of ~9 taps, per src window w): one batched dma_gather of
    x rows (fp32, 256B/row) for all round-taps; PE-transpose to channel-major;
    bf16 GEMM per tap against block-diag [[Wk,0],[0,Wk]] (2 rows per column,
    pair-packed so each column is tap-pure); PE-transpose back; DENSE write to
    an HBM contribution buffer C_r (bf16 rows padded to 128ch = 256B).
  Phase B (per round): dma_gather C_r rows in dst-major slot order (shared
    slot layout: each 128-dst block q owns max-over-cores slots, mult of 64);
    per 128-slot chunk one/two matmuls with the gathered chunk as stationary
    and a host-built 0/1 segment matrix streamed as rhs; accumulates h^T
    [64ch x 128dst] blocks in PSUM (memset + has_written semantics), added
    into an SBUF-resident hT [128, 12800] (even dst-chunk -> partitions 0:64,
    odd -> 64:128).
  Center tap: dense, feeds hT directly (no back-transpose).
  BN tail: free-dim reduces on hT, parity fold, 8-core AllReduce, fused
  Lrelu(h*s+b) on ACT, PE transpose back, y writes.
"""
import os
import numpy as np
import ml_dtypes

import concourse.bass as bass
import concourse.mybir as mybir
import concourse.bacc as bacc
import concourse.tile as tile
from concourse import bass_utils
from concourse.masks import make_identity

N = 200000
C = 8
V = N // C            # 25000 dst voxels per core
VP = 25600            # padded dst count (200 chunks of 128)
D = 64
K = 27
KC = 13               # center tap
WIN = 32768
NW = (N + WIN - 1) // WIN
NR = 3                # tap rounds
EPS = 1e-5
NEG = 0.01
F32 = mybir.dt.float32
BF16 = mybir.dt.bfloat16
I16 = mybir.dt.int16
NQ = 1                # SWDGE queues (multi-queue gave no overlap on HW)
BF = ml_dtypes.bfloat16
K2 = int(os.environ.get("K2PHASE", "4")) if os.environ.get("KERNEL_DEBUG") else 4


def _pack16(slab, col0, vals):
    w = vals.reshape(-1, 16).T
    L16 = w.shape[1]
    for r in range(8):
        slab[r * 16:(r + 1) * 16, col0:col0 + L16] = w
    return col0 + L16


def _prep_host(nbr):
    nbr = np.asarray(nbr, np.int64)
    taps = [k for k in range(K) if k != KC]
    rounds = [taps[0:9], taps[9:18], taps[18:26]]

    # per (c,k,w): (src_rel int32, dst_local int32) sorted by dst
    lists = {}
    for c in range(C):
        for k in taps:
            src = nbr[k, c * V:(c + 1) * V]
            valid = np.nonzero(src >= 0)[0]
            s = src[valid]
            for w in range(NW):
                m = (s >= w * WIN) & (s < (w + 1) * WIN)
                lists[(c, k, w)] = (s[m] - w * WIN, valid[m])

    # ---------- Phase A plan ----------
    # per (r,w): taps get column ranges (shared); positions = 2*cols
    aplan = []           # [r][w] -> dict(pos0, npos, tapcols=[(k,c0,c1)], ga0)
    CR = []              # C rows per round
    ga = 0
    for r, rtaps in enumerate(rounds):
        wplans = []
        pos0 = 0
        for w in range(NW):
            tapcols = []
            c0 = 0
            for k in rtaps:
                ncol = max((len(lists[(c, k, w)][0]) + 1) // 2 for c in range(C))
                tapcols.append((k, c0, c0 + ncol))
                c0 += ncol
            ncols = (c0 + 127) & ~127
            npos = 2 * ncols
            wplans.append(dict(pos0=pos0, npos=npos, tapcols=tapcols, ga0=ga))
            pos0 += npos
            ga += npos
        assert pos0 < 32768, f"round {r} C rows {pos0} exceed int16"
        aplan.append(wplans)
        CR.append(pos0)
    GA = ga

    # A idx slab + C positions of every contribution
    gaslab = np.zeros((C, 128, GA // 16), np.int16)
    cpos = {}            # (c,k,w) -> np.array of C positions (per round space)
    for c in range(C):
        idxA = np.zeros(GA, np.int16)
        for r in range(NR):
            for w in range(NW):
                wp = aplan[r][w]
                for (k, k0, k1) in wp["tapcols"]:
                    srcs, dsts = lists[(c, k, w)]
                    n = len(srcs)
                    j = np.arange(n)
                    col = k0 + j // 2
                    pos = 256 * (col // 128) + (col % 128) + 128 * (j % 2)
                    idxA[wp["ga0"] + pos] = srcs.astype(np.int16)
                    cpos[(c, k, w)] = wp["pos0"] + pos
        _pack16(gaslab[c], 0, idxA)

    # ---------- Phase B plan ----------
    # per round: per-core contributions (dst, cpos); shared slot layout per
    # 128-dst block q with L_q multiple of 64.
    NQB = VP // 128      # 200 dst chunks
    bplan = []           # [r] -> dict(gb0, nslots, chunks=[(s0,s1,q) subblocks])
    gb0 = 0
    percore_bc = {}      # (c, r) -> (dst array, cpos array) sorted by dst
    for r, rtaps in enumerate(rounds):
        nq = np.zeros((C, NQB), np.int64)
        for c in range(C):
            ds, ps = [], []
            for w in range(NW):
                for k in rtaps:
                    srcs, dsts = lists[(c, k, w)]
                    ds.append(dsts)
                    ps.append(cpos[(c, k, w)])
            ds = np.concatenate(ds)
            ps = np.concatenate(ps)
            o = np.argsort(ds, kind="stable")
            ds, ps = ds[o], ps[o]
            percore_bc[(c, r)] = (ds, ps)
            np.add.at(nq[c], ds // 128, 1)
        # L_q multiple of 128 so every 128-slot chunk maps to exactly one q
        # (partition-offset stationaries fault the PE — full-128 MMs only)
        Lq = ((nq.max(axis=0) + 127) // 128) * 128
        Sq = np.zeros(NQB + 1, np.int64)
        Sq[1:] = np.cumsum(Lq)
        nslots = int(Sq[-1])
        # chunk ci (128 slots) -> its dst block q
        qidx = np.searchsorted(Sq, np.arange(0, nslots, 128), side="right") - 1
        bplan.append(dict(gb0=gb0, nslots=nslots, chunks=qidx.tolist(), Sq=Sq,
                          Lq=Lq))
        gb0 += nslots
    GB = gb0

    gbslab = np.zeros((C, 128, GB // 16), np.int16)
    segslab = np.zeros((C, 128, GB), np.uint16)   # bf16 bit patterns
    ONE = np.float32(1.0).astype(BF).view(np.uint16)
    for c in range(C):
        idxB = np.zeros(GB, np.int16)
        for r in range(NR):
            bp = bplan[r]
            ds, ps = percore_bc[(c, r)]
            Sq = bp["Sq"]
            # slot index per contribution: Sq[q] + rank within q
            qv = ds // 128
            # ranks: ds sorted so within q contributions are consecutive
            starts = np.searchsorted(qv, np.arange(NQB))
            rank = np.arange(len(ds)) - starts[qv]
            slot = Sq[qv] + rank
            idxB[bp["gb0"] + slot] = ps.astype(np.int16)
            # seg: partition = slot%128, col = (slot//128)*128 + (dst%128)
            p = slot % 128
            col = bp["gb0"] + (slot // 128) * 128 + (ds % 128)
            segslab[c][p, col] = ONE
        _pack16(gbslab[c], 0, idxB)

    return dict(aplan=aplan, bplan=bplan, CR=CR, GA=GA, GB=GB), \
        gaslab, gbslab, segslab.view(BF)


# ----------------------------------------------------------------------------
_CURRENT = None


def _build_body(nc):
    plan = _CURRENT
    aplan, bplan, CR = plan["aplan"], plan["bplan"], plan["CR"]
    GA, GB = plan["GA"], plan["GB"]

    x_d = nc.dram_tensor("x_d", [N, D], F32, kind="ExternalInput")
    xc_d = nc.dram_tensor("xc_d", [VP, D], BF16, kind="ExternalInput")
    W_d = nc.dram_tensor("W_d", [K, D, D], BF16, kind="ExternalInput")
    gam_d = nc.dram_tensor("gam_d", [1, D], F32, kind="ExternalInput")
    bet_d = nc.dram_tensor("bet_d", [1, D], F32, kind="ExternalInput")
    ga_d = nc.dram_tensor("ga_d", [128, GA // 16], I16, kind="ExternalInput")
    gb_d = nc.dram_tensor("gb_d", [128, GB // 16], I16, kind="ExternalInput")
    seg_d = nc.dram_tensor("seg_d", [128, GB], BF16, kind="ExternalInput")
    y_d = nc.dram_tensor("y_d", [VP, D], F32, kind="ExternalOutput")

    with tile.TileContext(nc) as tc:
        with tc.tile_pool(name="sb", bufs=1) as sb, \
             tc.tile_pool(name="io", bufs=3) as io, \
             tc.tile_pool(name="ps", bufs=2, space="PSUM") as ps, \
             tc.tile_pool(name="dram", bufs=1, space="DRAM") as dram:

            c_bufs = [dram.tile([CR[r] + 256, 128], BF16, name=f"cbuf{r}")
                      for r in range(NR)]

            idf = sb.tile([128, 128], F32)
            make_identity(nc, idf[:])
            idb = sb.tile([128, 128], BF16)
            make_identity(nc, idb[:])

            ga_t = sb.tile([128, GA // 16], I16)
            nc.sync.dma_start(ga_t[:], ga_d[:, :])
            gb_t = sb.tile([128, GB // 16], I16)
            nc.sync.dma_start(gb_t[:], gb_d[:, :])

            W2 = sb.tile([128, K * 128], BF16)
            nc.gpsimd.memset(W2[:], 0.0)
            for k in range(K):
                nc.sync.dma_start(W2[0:D, k * 128:k * 128 + D], W_d[k, :, :])
                nc.sync.dma_start(W2[D:128, k * 128 + D:(k + 1) * 128], W_d[k, :, :])

            zt = sb.tile([128, 512], F32)
            nc.gpsimd.memset(zt[:], 0.0)

            hT = sb.tile([128, VP // 2], F32)   # [128, 12800]

            # ---- center tap: xc (m p) c view, transpose, MM, store to hT ----
            for g in range(25):
                xg = io.tile([128, 512], BF16, tag="xg")
                nc.sync.dma_start(
                    xg[:].rearrange("p (m c) -> p m c", m=8),
                    xc_d[1024 * g:1024 * (g + 1), :].rearrange(
                        "(m p) c -> p m c", m=8, p=128))
                pa = ps.tile([128, 512], BF16, tag="psAb", space="PSUM", bufs=1)
                for j in range(4):
                    nc.tensor.transpose(out=pa[:, j * 128:(j + 1) * 128],
                                        in_=xg[:, j * 128:(j + 1) * 128],
                                        identity=idb[:])
                ct = io.tile([128, 512], BF16, tag="ct")
                nc.vector.tensor_copy(ct[:], pa[:])
                pc = ps.tile([128, 512], F32, tag="psC", space="PSUM")
                nc.tensor.matmul(out=pc[:], lhsT=W2[:, KC * 128:(KC + 1) * 128],
                                 rhs=ct[:], start=True, stop=True)
                nc.vector.tensor_copy(hT[:, g * 512:(g + 1) * 512], pc[:])

            # ---- phase A: per (round, window) batched gather -> C_r ----
            for r in range(NR if K2 >= 2 else 0):
                for w in range(NW):
                    wp = aplan[r][w]
                    npos = wp["npos"]
                    if npos == 0:
                        continue
                    nslot = npos // 128
                    assert nslot <= 48, nslot
                    gbuf = io.tile([128, 48, D], F32, tag="gbuf")
                    nc.gpsimd.dma_gather(
                        out_ap=gbuf[:, 0:nslot, :],
                        in_ap=x_d[w * WIN:min(N, (w + 1) * WIN), :],
                        idxs_ap=ga_t[:, wp["ga0"] // 16:(wp["ga0"] + npos) // 16],
                        num_idxs=npos, num_idxs_reg=npos, elem_size=D,
                        single_packet=False, queue_num=(r * NW + w) % NQ)
                    gbf = gbuf[:, 0:nslot, :].rearrange("p m d -> p (m d)")
                    ncols = npos // 2
                    for g0 in range(0, ncols, 512):
                        gw = min(512, ncols - g0)
                        pa = ps.tile([128, 512], F32, tag="psA", space="PSUM")
                        for j in range(gw // 128):
                            nc.tensor.transpose(
                                out=pa[:, j * 128:(j + 1) * 128],
                                in_=gbf[:, g0 + j * 128:g0 + (j + 1) * 128],
                                identity=idf[:])
                        ct = io.tile([128, 512], BF16, tag="ct")
                        nc.vector.tensor_copy(ct[:, 0:gw], pa[:, 0:gw])
                        pb = ps.tile([128, 512], F32, tag="psB", space="PSUM")
                        for (k, k0, k1) in wp["tapcols"]:
                            a, b = max(k0, g0), min(k1, g0 + gw)
                            if b <= a:
                                continue
                            nc.tensor.matmul(
                                out=pb[:, a - g0:b - g0],
                                lhsT=W2[:, k * 128:(k + 1) * 128],
                                rhs=ct[:, a - g0:b - g0], start=True, stop=True)
                        hb = io.tile([128, 512], BF16, tag="hb")
                        nc.vector.tensor_copy(hb[:, 0:gw], pb[:, 0:gw])
                        pc = ps.tile([128, 512], BF16, tag="psCb", space="PSUM", bufs=1)
                        for j in range(gw // 128):
                            nc.tensor.transpose(
                                out=pc[:, j * 128:(j + 1) * 128],
                                in_=hb[:, j * 128:(j + 1) * 128],
                                identity=idb[:])
                        so = io.tile([128, 512], BF16, tag="so")
                        nc.scalar.activation(so[:, 0:gw], pc[:, 0:gw],
                                             mybir.ActivationFunctionType.Copy,
                                             bias=0.0)
                        p0 = wp["pos0"] + 2 * g0
                        nc.sync.dma_start(
                            c_bufs[r][p0:p0 + 2 * gw, 0:64].rearrange(
                                "(m p) c -> p m c", p=128),
                            so[:, 0:gw].rearrange("p (m c) -> p m c", c=64))

            # ---- phase B: per round dst-major gather + seg matmuls ----
            NG = VP // 1024    # 25 psum groups of 1024 dsts
            for r in range(NR if K2 >= 3 else 0):
                bp = bplan[r]
                nslots = bp["nslots"]
                nchunk = nslots // 128
                chunks = bp["chunks"]
                # sub-gathers of up to 32 chunks
                SUB = 32
                subs = [(s, min(s + SUB, nchunk)) for s in range(0, nchunk, SUB)]
                gtiles = {}
                for si, (c0, c1) in enumerate(subs):
                    g2 = io.tile([128, SUB, 128], BF16, tag="g2")
                    nc.gpsimd.dma_gather(
                        out_ap=g2[:, 0:c1 - c0, :],
                        in_ap=c_bufs[r][:, :],
                        idxs_ap=gb_t[:, (bp["gb0"] + c0 * 128) // 16:
                                     (bp["gb0"] + c1 * 128) // 16],
                        num_idxs=(c1 - c0) * 128, num_idxs_reg=(c1 - c0) * 128,
                        elem_size=128, single_packet=False)
                    g2f = g2[:].rearrange("p m d -> p (m d)")
                    for ci in range(c0, c1):
                        gtiles[ci] = (g2f, ci - c0)
                # seg stream tiles of 16 chunks (2048 cols)
                segtiles = {}
                for s0 in range(0, nchunk, 16):
                    s1 = min(s0 + 16, nchunk)
                    st = io.tile([128, 16 * 128], BF16, tag="st")
                    nc.sync.dma_start(
                        st[:, 0:(s1 - s0) * 128],
                        seg_d[:, bp["gb0"] + s0 * 128:bp["gb0"] + s1 * 128])
                    for ci in range(s0, s1):
                        segtiles[ci] = (st, ci - s0)
                # psum groups
                for t in range(NG):
                    pm = ps.tile([128, 512], F32, tag="psA", space="PSUM")
                    nc.vector.tensor_copy(pm[:, :], zt[:, :])
                    qlo, qhi = 8 * t, 8 * t + 8
                    for ci in range(nchunk):
                        q = chunks[ci]
                        if q < qlo or q >= qhi:
                            continue
                        g2f, m = gtiles[ci]
                        st, sc = segtiles[ci]
                        ro = 0 if q % 2 == 0 else 64
                        cb = ((q - qlo) // 2) * 128
                        nc.tensor.matmul(
                            out=pm[ro:ro + 64, cb:cb + 128],
                            lhsT=g2f[0:128, m * 128:m * 128 + 64],
                            rhs=st[0:128, sc * 128:(sc + 1) * 128],
                            start=False, stop=True)
                    cols = slice(512 * t, 512 * (t + 1))
                    nc.vector.tensor_tensor(out=hT[:, cols], in0=pm[:, :],
                                            in1=hT[:, cols],
                                            op=mybir.AluOpType.add)

            # ---- BN stats on hT ----
            sacc = sb.tile([128, 32], F32)
            qacc = sb.tile([128, 32], F32)
            nc.gpsimd.memset(sacc[:], 0.0)
            nc.gpsimd.memset(qacc[:], 0.0)
            for g in range(25):
                ch = hT[:, g * 512:(g + 1) * 512]
                nc.vector.tensor_reduce(out=sacc[:, g:g + 1], in_=ch,
                                        axis=mybir.AxisListType.X,
                                        op=mybir.AluOpType.add)
                scr = io.tile([128, 512], F32, tag="scr")
                nc.vector.tensor_tensor(out=scr[:], in0=ch, in1=ch,
                                        op=mybir.AluOpType.mult)
                nc.vector.tensor_reduce(out=qacc[:, g:g + 1], in_=scr[:],
                                        axis=mybir.AxisListType.X,
                                        op=mybir.AluOpType.add)
            sq = sb.tile([128, 2], F32)
            nc.vector.tensor_reduce(out=sq[:, 0:1], in_=sacc[:, 0:25],
                                    axis=mybir.AxisListType.X,
                                    op=mybir.AluOpType.add)
            nc.vector.tensor_reduce(out=sq[:, 1:2], in_=qacc[:, 0:25],
                                    axis=mybir.AxisListType.X,
                                    op=mybir.AluOpType.add)
            sqh = sb.tile([64, 2], F32)
            nc.sync.dma_start(sqh[:], sq[64:128, :])
            sq64 = sb.tile([64, 2], F32)
            nc.vector.tensor_tensor(out=sq64[:], in0=sq[0:64, :], in1=sqh[:],
                                    op=mybir.AluOpType.add)
            cc_in = dram.tile([64, 2], F32)
            cc_out = dram.tile([64, 2], F32)
            nc.gpsimd.dma_start(cc_in[:], sq64[:])
            nc.gpsimd.collective_compute(
                "AllReduce", mybir.AluOpType.add,
                replica_groups=[list(range(C))],
                ins=[cc_in.opt()], outs=[cc_out.opt()])
            g2r = sb.tile([64, 2], F32)
            nc.sync.dma_start(g2r[:], cc_out[:])
            me = sb.tile([64, 2], F32)
            nc.vector.tensor_scalar_mul(me[:], g2r[:], 1.0 / N)
            v1 = sb.tile([64, 1], F32)
            nc.vector.tensor_tensor(out=v1[:], in0=me[:, 0:1], in1=me[:, 0:1],
                                    op=mybir.AluOpType.mult)
            nc.vector.tensor_tensor(out=v1[:], in0=me[:, 1:2], in1=v1[:],
                                    op=mybir.AluOpType.subtract)
            eps_t = sb.tile([64, 1], F32)
            nc.gpsimd.memset(eps_t[:], EPS)
            std = sb.tile([64, 1], F32)
            nc.scalar.activation(std[:], v1[:], mybir.ActivationFunctionType.Sqrt,
                                 bias=eps_t[:])
            rin = sb.tile([64, 1], F32)
            nc.vector.reciprocal(rin[:], std[:])
            gam = sb.tile([64, 1], F32)
            nc.sync.dma_start(gam[:], gam_d[0, :, None])
            bet = sb.tile([64, 1], F32)
            nc.sync.dma_start(bet[:], bet_d[0, :, None])
            sc_h = sb.tile([64, 1], F32)
            nc.vector.tensor_tensor(out=sc_h[:], in0=rin[:], in1=gam[:],
                                    op=mybir.AluOpType.mult)
            cb_h = sb.tile([64, 1], F32)
            nc.vector.tensor_tensor(out=cb_h[:], in0=me[:, 0:1], in1=sc_h[:],
                                    op=mybir.AluOpType.mult)
            nc.vector.tensor_tensor(out=cb_h[:], in0=bet[:], in1=cb_h[:],
                                    op=mybir.AluOpType.subtract)
            s128 = sb.tile([128, 2], F32)
            nc.sync.dma_start(s128[0:64, 0:1], sc_h[:])
            nc.sync.dma_start(s128[64:128, 0:1], sc_h[:])
            nc.sync.dma_start(s128[0:64, 1:2], cb_h[:])
            nc.sync.dma_start(s128[64:128, 1:2], cb_h[:])

            # ---- apply + transpose back + write y ----
            for g in range(25):
                ha = io.tile([128, 512], F32, tag="ha")
                nc.scalar.activation(ha[:], hT[:, g * 512:(g + 1) * 512],
                                     mybir.ActivationFunctionType.Lrelu,
                                     bias=s128[:, 1:2], scale=s128[:, 0:1],
                                     alpha=NEG)
                pc = ps.tile([128, 512], F32, tag="psC", space="PSUM")
                for j in range(4):
                    nc.tensor.transpose(out=pc[:, j * 128:(j + 1) * 128],
                                        in_=ha[:, j * 128:(j + 1) * 128],
                                        identity=idf[:])
                yo = io.tile([128, 512], F32, tag="yo")
                nc.vector.tensor_copy(yo[:], pc[:])
                nc.sync.dma_start(
                    y_d[1024 * g:1024 * (g + 1), :].rearrange(
                        "(a p) c -> p a c", a=8, p=128),
                    yo[:].rearrange("p (a c) -> p a c", c=64))


def _build_program(plan):
    global _CURRENT
    _CURRENT = plan
    nc = bacc.Bacc("TRN2", target_bir_lowering=False, debug=False,
                   num_devices=C, num_swdge_queues=NQ)
    _build_body(nc)
    nc.compile()
    return nc


_CACHE = {}


def build(nbr):
    key = nbr.tobytes()[:4096] + nbr.tobytes()[-4096:]
    if key in _CACHE:
        return _CACHE[key]
    plan, gaslab, gbslab, segslab = _prep_host(np.asarray(nbr))
    nc = _build_program(plan)
    _CACHE[key] = (nc, gaslab, gbslab, segslab)
    return _CACHE[key]


def kernel(x, W, gamma, beta, nbr):
    x = np.ascontiguousarray(np.asarray(x, np.float32))
    Wb = np.asarray(W, np.float32).astype(BF)
    gamma = np.asarray(gamma, np.float32).reshape(1, D)
    beta = np.asarray(beta, np.float32).reshape(1, D)
    nbr = np.asarray(nbr)
    nc, gaslab, gbslab, segslab = build(nbr)
    in_maps = []
    for c in range(C):
        xc = np.zeros((VP, D), BF)
        xc[:V] = x[c * V:(c + 1) * V].astype(BF)
        in_maps.append({
            "x_d": x,
            "xc_d": xc,
            "W_d": Wb,
            "gam_d": gamma,
            "bet_d": beta,
            "ga_d": gaslab[c],
            "gb_d": gbslab[c],
            "seg_d": segslab[c],
        })
    res = bass_utils.run_bass_kernel_spmd(nc, in_maps, core_ids=list(range(C)))
    return np.concatenate([res.results[c]["y_d"][:V] for c in range(C)], axis=0)


# revision 20
# speedup vs baseline: 3.0604x; 1.2331x over previous
"""Sparse transposed-conv (27-tap) + BN + LeakyReLU on 8 TRN2 cores — v2.

Scatter-free design (the baseline's dma_scatter_add CCE-RMW was ~1.7ms):
  Phase A (per round r of ~9 taps, per src window w): one batched dma_gather of
    x rows (fp32, 256B/row) for all round-taps; PE-transpose to channel-major;
    bf16 GEMM per tap against block-diag [[Wk,0],[0,Wk]] (2 rows per column,
    pair-packed so each column is tap-pure); PE-transpose back; DENSE write to
    an HBM contribution buffer C_r (bf16 rows padded to 128ch = 256B).
  Phase B (per round): dma_gather C_r rows in dst-major slot order (shared
    slot layout: each 128-dst block q owns max-over-cores slots, mult of 64);
    per 128-slot chunk one/two matmuls with the gathered chunk as stationary
    and a host-built 0/1 segment matrix streamed as rhs; accumulates h^T
    [64ch x 128dst] blocks in PSUM (memset + has_written semantics), added
    into an SBUF-resident hT [128, 12800] (even dst-chunk -> partitions 0:64,
    odd -> 64:128).
  Center tap: dense, feeds hT directly (no back-transpose).
  BN tail: free-dim reduces on hT, parity fold, 8-core AllReduce, fused
  Lrelu(h*s+b) on ACT, PE transpose back, y writes.
"""
import os
import numpy as np
import ml_dtypes

import concourse.bass as bass
import concourse.mybir as mybir
import concourse.bacc as bacc
import concourse.tile as tile
from concourse import bass_utils
from concourse.masks import make_identity

N = 200000
C = 8
V = N // C            # 25000 dst voxels per core
VP = 25600            # padded dst count (200 chunks of 128)
D = 64
K = 27
KC = 13               # center tap
WIN = 32768
NW = (N + WIN - 1) // WIN
NR = 3                # tap rounds
EPS = 1e-5
NEG = 0.01
F32 = mybir.dt.float32
BF16 = mybir.dt.bfloat16
I16 = mybir.dt.int16
NQ = 1                # SWDGE queues (multi-queue gave no overlap on HW)
BF = ml_dtypes.bfloat16
K2 = int(os.environ.get("K2PHASE", "4")) if os.environ.get("KERNEL_DEBUG") else 4


def _pack16(slab, col0, vals):
    w = vals.reshape(-1, 16).T
    L16 = w.shape[1]
    for r in range(8):
        slab[r * 16:(r + 1) * 16, col0:col0 + L16] = w
    return col0 + L16


def _prep_host(nbr):
    nbr = np.asarray(nbr, np.int64)
    taps = [k for k in range(K) if k != KC]
    rounds = [taps[0:9], taps[9:18], taps[18:26]]

    # per (c,k,w): (src_rel int32, dst_local int32) sorted by dst
    lists = {}
    for c in range(C):
        for k in taps:
            src = nbr[k, c * V:(c + 1) * V]
            valid = np.nonzero(src >= 0)[0]
            s = src[valid]
            for w in range(NW):
                m = (s >= w * WIN) & (s < (w + 1) * WIN)
                lists[(c, k, w)] = (s[m] - w * WIN, valid[m])

    # ---------- Phase A plan ----------
    # per (r,w): taps get column ranges (shared); positions = 2*cols
    aplan = []           # [r][w] -> dict(pos0, npos, tapcols=[(k,c0,c1)], ga0)
    CR = []              # C rows per round
    for r, rtaps in enumerate(rounds):
        wplans = []
        pos0 = 0
        for w in range(NW):
            tapcols = []
            c0 = 0
            for k in rtaps:
                ncol = max((len(lists[(c, k, w)][0]) + 1) // 2 for c in range(C))
                tapcols.append((k, c0, c0 + ncol))
                c0 += ncol
            ncols = (c0 + 127) & ~127
            npos = 2 * ncols
            wplans.append(dict(pos0=pos0, npos=npos, tapcols=tapcols, ga0=0))
            pos0 += npos
        assert pos0 < 32768, f"round {r} C rows {pos0} exceed int16"
        aplan.append(wplans)
        CR.append(pos0)
    # slab offsets in (window, round) order so one gather per window covers
    # all rounds' segments contiguously
    ga = 0
    for w in range(NW):
        for r in range(NR):
            aplan[r][w]["ga0"] = ga
            ga += aplan[r][w]["npos"]
    GA = ga

    # A idx slab + C positions of every contribution
    gaslab = np.zeros((C, 128, GA // 16), np.int16)
    cpos = {}            # (c,k,w) -> np.array of C positions (per round space)
    for c in range(C):
        idxA = np.zeros(GA, np.int16)
        for r in range(NR):
            for w in range(NW):
                wp = aplan[r][w]
                for (k, k0, k1) in wp["tapcols"]:
                    srcs, dsts = lists[(c, k, w)]
                    n = len(srcs)
                    j = np.arange(n)
                    col = k0 + j // 2
                    pos = 256 * (col // 128) + (col % 128) + 128 * (j % 2)
                    idxA[wp["ga0"] + pos] = srcs.astype(np.int16)
                    cpos[(c, k, w)] = wp["pos0"] + pos
        _pack16(gaslab[c], 0, idxA)

    # ---------- Phase B plan ----------
    # per round: per-core contributions (dst, cpos); shared slot layout per
    # 128-dst block q with L_q multiple of 64.
    NQB = VP // 128      # 200 dst chunks
    bplan = []           # [r] -> dict(gb0, nslots, chunks=[(s0,s1,q) subblocks])
    gb0 = 0
    percore_bc = {}      # (c, r) -> (dst array, cpos array) sorted by dst
    for r, rtaps in enumerate(rounds):
        nq = np.zeros((C, NQB), np.int64)
        for c in range(C):
            ds, ps = [], []
            for w in range(NW):
                for k in rtaps:
                    srcs, dsts = lists[(c, k, w)]
                    ds.append(dsts)
                    ps.append(cpos[(c, k, w)])
            ds = np.concatenate(ds)
            ps = np.concatenate(ps)
            o = np.argsort(ds, kind="stable")
            ds, ps = ds[o], ps[o]
            percore_bc[(c, r)] = (ds, ps)
            np.add.at(nq[c], ds // 128, 1)
        # L_q multiple of 128 so every 128-slot chunk maps to exactly one q
        # (partition-offset stationaries fault the PE — full-128 MMs only)
        Lq = ((nq.max(axis=0) + 127) // 128) * 128
        Sq = np.zeros(NQB + 1, np.int64)
        Sq[1:] = np.cumsum(Lq)
        nslots = int(Sq[-1])
        # chunk ci (128 slots) -> its dst block q
        qidx = np.searchsorted(Sq, np.arange(0, nslots, 128), side="right") - 1
        bplan.append(dict(gb0=gb0, nslots=nslots, chunks=qidx.tolist(), Sq=Sq,
                          Lq=Lq))
        gb0 += nslots
    GB = gb0

    gbslab = np.zeros((C, 128, GB // 16), np.int16)
    segslab = np.zeros((C, 128, GB), np.uint16)   # bf16 bit patterns
    ONE = np.float32(1.0).astype(BF).view(np.uint16)
    for c in range(C):
        idxB = np.zeros(GB, np.int16)
        for r in range(NR):
            bp = bplan[r]
            ds, ps = percore_bc[(c, r)]
            Sq = bp["Sq"]
            # slot index per contribution: Sq[q] + rank within q
            qv = ds // 128
            # ranks: ds sorted so within q contributions are consecutive
            starts = np.searchsorted(qv, np.arange(NQB))
            rank = np.arange(len(ds)) - starts[qv]
            slot = Sq[qv] + rank
            idxB[bp["gb0"] + slot] = ps.astype(np.int16)
            # seg: partition = slot%128, col = (slot//128)*128 + (dst%128)
            p = slot % 128
            col = bp["gb0"] + (slot // 128) * 128 + (ds % 128)
            segslab[c][p, col] = ONE
        _pack16(gbslab[c], 0, idxB)

    return dict(aplan=aplan, bplan=bplan, CR=CR, GA=GA, GB=GB), \
        gaslab, gbslab, segslab.view(BF)


# ----------------------------------------------------------------------------
_CURRENT = None


def _build_body(nc):
    plan = _CURRENT
    aplan, bplan, CR = plan["aplan"], plan["bplan"], plan["CR"]
    GA, GB = plan["GA"], plan["GB"]

    x_d = nc.dram_tensor("x_d", [N, D], F32, kind="ExternalInput")
    xc_d = nc.dram_tensor("xc_d", [VP, D], BF16, kind="ExternalInput")
    W_d = nc.dram_tensor("W_d", [K, D, D], BF16, kind="ExternalInput")
    gam_d = nc.dram_tensor("gam_d", [1, D], F32, kind="ExternalInput")
    bet_d = nc.dram_tensor("bet_d", [1, D], F32, kind="ExternalInput")
    ga_d = nc.dram_tensor("ga_d", [128, GA // 16], I16, kind="ExternalInput")
    gb_d = nc.dram_tensor("gb_d", [128, GB // 16], I16, kind="ExternalInput")
    seg_d = nc.dram_tensor("seg_d", [128, GB], BF16, kind="ExternalInput")
    y_d = nc.dram_tensor("y_d", [VP, D], F32, kind="ExternalOutput")

    with tile.TileContext(nc) as tc:
        with tc.tile_pool(name="sb", bufs=1) as sb, \
             tc.tile_pool(name="io", bufs=3) as io, \
             tc.tile_pool(name="ps", bufs=2, space="PSUM") as ps, \
             tc.tile_pool(name="dram", bufs=1, space="DRAM") as dram:

            c_bufs = [dram.tile([CR[r] + 256, 128], BF16, name=f"cbuf{r}")
                      for r in range(NR)]

            idf = sb.tile([128, 128], F32)
            make_identity(nc, idf[:])
            idb = sb.tile([128, 128], BF16)
            make_identity(nc, idb[:])

            ga_t = sb.tile([128, GA // 16], I16)
            nc.sync.dma_start(ga_t[:], ga_d[:, :])
            gb_t = sb.tile([128, GB // 16], I16)
            nc.sync.dma_start(gb_t[:], gb_d[:, :])

            W2 = sb.tile([128, K * 128], BF16)
            nc.gpsimd.memset(W2[:], 0.0)
            for k in range(K):
                nc.sync.dma_start(W2[0:D, k * 128:k * 128 + D], W_d[k, :, :])
                nc.sync.dma_start(W2[D:128, k * 128 + D:(k + 1) * 128], W_d[k, :, :])

            zt = sb.tile([128, 512], F32)
            nc.gpsimd.memset(zt[:], 0.0)

            hT = sb.tile([128, VP // 2], F32)   # [128, 12800]

            # ---- center tap: xc (m p) c view, transpose, MM, store to hT ----
            for g in range(25):
                xg = io.tile([128, 512], BF16, tag="xg")
                nc.sync.dma_start(
                    xg[:].rearrange("p (m c) -> p m c", m=8),
                    xc_d[1024 * g:1024 * (g + 1), :].rearrange(
                        "(m p) c -> p m c", m=8, p=128))
                pa = ps.tile([128, 512], BF16, tag="psAb", space="PSUM", bufs=1)
                for j in range(4):
                    nc.tensor.transpose(out=pa[:, j * 128:(j + 1) * 128],
                                        in_=xg[:, j * 128:(j + 1) * 128],
                                        identity=idb[:])
                ct = io.tile([128, 512], BF16, tag="ct")
                nc.vector.tensor_copy(ct[:], pa[:])
                pc = ps.tile([128, 512], F32, tag="psC", space="PSUM")
                nc.tensor.matmul(out=pc[:], lhsT=W2[:, KC * 128:(KC + 1) * 128],
                                 rhs=ct[:], start=True, stop=True)
                nc.vector.tensor_copy(hT[:, g * 512:(g + 1) * 512], pc[:])

            # ---- phase A: one gather per window (all rounds) -> C_r ----
            for w in range(NW if K2 >= 2 else 0):
                wps = [aplan[r][w] for r in range(NR)]
                tpos = sum(wp["npos"] for wp in wps)
                if tpos == 0:
                    continue
                tslot = tpos // 128
                assert tslot <= 96, tslot
                gbuf = io.tile([128, 96, D], F32, tag="gbuf", bufs=2)
                nc.gpsimd.dma_gather(
                    out_ap=gbuf[:, 0:tslot, :],
                    in_ap=x_d[w * WIN:min(N, (w + 1) * WIN), :],
                    idxs_ap=ga_t[:, wps[0]["ga0"] // 16:
                                 (wps[0]["ga0"] + tpos) // 16],
                    num_idxs=tpos, num_idxs_reg=tpos, elem_size=D,
                    single_packet=False)
                for r in range(NR):
                    wp = wps[r]
                    npos = wp["npos"]
                    if npos == 0:
                        continue
                    soff = (wp["ga0"] - wps[0]["ga0"]) // 128
                    gbf = gbuf[:, soff:soff + npos // 128, :].rearrange(
                        "p m d -> p (m d)")
                    ncols = npos // 2
                    for g0 in range(0, ncols, 512):
                        gw = min(512, ncols - g0)
                        pa = ps.tile([128, 512], F32, tag="psA", space="PSUM")
                        for j in range(gw // 128):
                            nc.tensor.transpose(
                                out=pa[:, j * 128:(j + 1) * 128],
                                in_=gbf[:, g0 + j * 128:g0 + (j + 1) * 128],
                                identity=idf[:])
                        ct = io.tile([128, 512], BF16, tag="ct")
                        nc.vector.tensor_copy(ct[:, 0:gw], pa[:, 0:gw])
                        pb = ps.tile([128, 512], F32, tag="psB", space="PSUM")
                        for (k, k0, k1) in wp["tapcols"]:
                            a, b = max(k0, g0), min(k1, g0 + gw)
                            if b <= a:
                                continue
                            nc.tensor.matmul(
                                out=pb[:, a - g0:b - g0],
                                lhsT=W2[:, k * 128:(k + 1) * 128],
                                rhs=ct[:, a - g0:b - g0], start=True, stop=True)
                        hb = io.tile([128, 512], BF16, tag="hb")
                        nc.vector.tensor_copy(hb[:, 0:gw], pb[:, 0:gw])
                        pc = ps.tile([128, 512], BF16, tag="psCb", space="PSUM", bufs=1)
                        for j in range(gw // 128):
                            nc.tensor.transpose(
                                out=pc[:, j * 128:(j + 1) * 128],
                                in_=hb[:, j * 128:(j + 1) * 128],
                                identity=idb[:])
                        so = io.tile([128, 512], BF16, tag="so")
                        nc.scalar.activation(so[:, 0:gw], pc[:, 0:gw],
                                             mybir.ActivationFunctionType.Copy,
                                             bias=0.0)
                        p0 = wp["pos0"] + 2 * g0
                        nc.sync.dma_start(
                            c_bufs[r][p0:p0 + 2 * gw, 0:64].rearrange(
                                "(m p) c -> p m c", p=128),
                            so[:, 0:gw].rearrange("p (m c) -> p m c", c=64))

            # ---- phase B: per round dst-major gather + seg matmuls ----
            NG = VP // 1024    # 25 psum groups of 1024 dsts
            for r in range(NR if K2 >= 3 else 0):
                bp = bplan[r]
                nslots = bp["nslots"]
                nchunk = nslots // 128
                chunks = bp["chunks"]
                # sub-gathers of up to 32 chunks
                SUB = 32
                subs = [(s, min(s + SUB, nchunk)) for s in range(0, nchunk, SUB)]
                gtiles = {}
                for si, (c0, c1) in enumerate(subs):
                    g2 = io.tile([128, SUB, 128], BF16, tag="g2")
                    nc.gpsimd.dma_gather(
                        out_ap=g2[:, 0:c1 - c0, :],
                        in_ap=c_bufs[r][:, :],
                        idxs_ap=gb_t[:, (bp["gb0"] + c0 * 128) // 16:
                                     (bp["gb0"] + c1 * 128) // 16],
                        num_idxs=(c1 - c0) * 128, num_idxs_reg=(c1 - c0) * 128,
                        elem_size=128, single_packet=False)
                    g2f = g2[:].rearrange("p m d -> p (m d)")
                    for ci in range(c0, c1):
                        gtiles[ci] = (g2f, ci - c0)
                # seg stream tiles of 16 chunks (2048 cols)
                segtiles = {}
                for s0 in range(0, nchunk, 16):
                    s1 = min(s0 + 16, nchunk)
                    st = io.tile([128, 16 * 128], BF16, tag="st")
                    nc.sync.dma_start(
                        st[:, 0:(s1 - s0) * 128],
                        seg_d[:, bp["gb0"] + s0 * 128:bp["gb0"] + s1 * 128])
                    for ci in range(s0, s1):
                        segtiles[ci] = (st, ci - s0)
                # psum groups
                for t in range(NG):
                    pm = ps.tile([128, 512], F32, tag="psA", space="PSUM")
                    nc.vector.tensor_copy(pm[:, :], zt[:, :])
                    qlo, qhi = 8 * t, 8 * t + 8
                    for ci in range(nchunk):
                        q = chunks[ci]
                        if q < qlo or q >= qhi:
                            continue
                        g2f, m = gtiles[ci]
                        st, sc = segtiles[ci]
                        ro = 0 if q % 2 == 0 else 64
                        cb = ((q - qlo) // 2) * 128
                        nc.tensor.matmul(
                            out=pm[ro:ro + 64, cb:cb + 128],
                            lhsT=g2f[0:128, m * 128:m * 128 + 64],
                            rhs=st[0:128, sc * 128:(sc + 1) * 128],
                            start=False, stop=True)
                    cols = slice(512 * t, 512 * (t + 1))
                    nc.vector.tensor_tensor(out=hT[:, cols], in0=pm[:, :],
                                            in1=hT[:, cols],
                                            op=mybir.AluOpType.add)

            # ---- BN stats on hT ----
            sacc = sb.tile([128, 32], F32)
            qacc = sb.tile([128, 32], F32)
            nc.gpsimd.memset(sacc[:], 0.0)
            nc.gpsimd.memset(qacc[:], 0.0)
            for g in range(25):
                ch = hT[:, g * 512:(g + 1) * 512]
                nc.vector.tensor_reduce(out=sacc[:, g:g + 1], in_=ch,
                                        axis=mybir.AxisListType.X,
                                        op=mybir.AluOpType.add)
                scr = io.tile([128, 512], F32, tag="scr")
                nc.vector.tensor_tensor(out=scr[:], in0=ch, in1=ch,
                                        op=mybir.AluOpType.mult)
                nc.vector.tensor_reduce(out=qacc[:, g:g + 1], in_=scr[:],
                                        axis=mybir.AxisListType.X,
                                        op=mybir.AluOpType.add)
            sq = sb.tile([128, 2], F32)
            nc.vector.tensor_reduce(out=sq[:, 0:1], in_=sacc[:, 0:25],
                                    axis=mybir.AxisListType.X,
                                    op=mybir.AluOpType.add)
            nc.vector.tensor_reduce(out=sq[:, 1:2], in_=qacc[:, 0:25],
                                    axis=mybir.AxisListType.X,
                                    op=mybir.AluOpType.add)
            sqh = sb.tile([64, 2], F32)
            nc.sync.dma_start(sqh[:], sq[64:128, :])
            sq64 = sb.tile([64, 2], F32)
            nc.vector.tensor_tensor(out=sq64[:], in0=sq[0:64, :], in1=sqh[:],
                                    op=mybir.AluOpType.add)
            cc_in = dram.tile([64, 2], F32)
            cc_out = dram.tile([64, 2], F32)
            nc.gpsimd.dma_start(cc_in[:], sq64[:])
            nc.gpsimd.collective_compute(
                "AllReduce", mybir.AluOpType.add,
                replica_groups=[list(range(C))],
                ins=[cc_in.opt()], outs=[cc_out.opt()])
            g2r = sb.tile([64, 2], F32)
            nc.sync.dma_start(g2r[:], cc_out[:])
            me = sb.tile([64, 2], F32)
            nc.vector.tensor_scalar_mul(me[:], g2r[:], 1.0 / N)
            v1 = sb.tile([64, 1], F32)
            nc.vector.tensor_tensor(out=v1[:], in0=me[:, 0:1], in1=me[:, 0:1],
                                    op=mybir.AluOpType.mult)
            nc.vector.tensor_tensor(out=v1[:], in0=me[:, 1:2], in1=v1[:],
                                    op=mybir.AluOpType.subtract)
            eps_t = sb.tile([64, 1], F32)
            nc.gpsimd.memset(eps_t[:], EPS)
            std = sb.tile([64, 1], F32)
            nc.scalar.activation(std[:], v1[:], mybir.ActivationFunctionType.Sqrt,
                                 bias=eps_t[:])
            rin = sb.tile([64, 1], F32)
            nc.vector.reciprocal(rin[:], std[:])
            gam = sb.tile([64, 1], F32)
            nc.sync.dma_start(gam[:], gam_d[0, :, None])
            bet = sb.tile([64, 1], F32)
            nc.sync.dma_start(bet[:], bet_d[0, :, None])
            sc_h = sb.tile([64, 1], F32)
            nc.vector.tensor_tensor(out=sc_h[:], in0=rin[:], in1=gam[:],
                                    op=mybir.AluOpType.mult)
            cb_h = sb.tile([64, 1], F32)
            nc.vector.tensor_tensor(out=cb_h[:], in0=me[:, 0:1], in1=sc_h[:],
                                    op=mybir.AluOpType.mult)
            nc.vector.tensor_tensor(out=cb_h[:], in0=bet[:], in1=cb_h[:],
                                    op=mybir.AluOpType.subtract)
            s128 = sb.tile([128, 2], F32)
            nc.sync.dma_start(s128[0:64, 0:1], sc_h[:])
            nc.sync.dma_start(s128[64:128, 0:1], sc_h[:])
            nc.sync.dma_start(s128[0:64, 1:2], cb_h[:])
            nc.sync.dma_start(s128[64:128, 1:2], cb_h[:])

            # ---- apply + transpose back + write y ----
            for g in range(25):
                ha = io.tile([128, 512], F32, tag="ha")
                nc.scalar.activation(ha[:], hT[:, g * 512:(g + 1) * 512],
                                     mybir.ActivationFunctionType.Lrelu,
                                     bias=s128[:, 1:2], scale=s128[:, 0:1],
                                     alpha=NEG)
                pc = ps.tile([128, 512], F32, tag="psC", space="PSUM")
                for j in range(4):
                    nc.tensor.transpose(out=pc[:, j * 128:(j + 1) * 128],
                                        in_=ha[:, j * 128:(j + 1) * 128],
                                        identity=idf[:])
                yo = io.tile([128, 512], F32, tag="yo")
                nc.vector.tensor_copy(yo[:], pc[:])
                nc.sync.dma_start(
                    y_d[1024 * g:1024 * (g + 1), :].rearrange(
                        "(a p) c -> p a c", a=8, p=128),
                    yo[:].rearrange("p (a c) -> p a c", c=64))


def _build_program(plan):
    global _CURRENT
    _CURRENT = plan
    nc = bacc.Bacc("TRN2", target_bir_lowering=False, debug=False,
                   num_devices=C, num_swdge_queues=NQ)
    _build_body(nc)
    nc.compile()
    return nc


_CACHE = {}


def build(nbr):
    key = nbr.tobytes()[:4096] + nbr.tobytes()[-4096:]
    if key in _CACHE:
        return _CACHE[key]
    plan, gaslab, gbslab, segslab = _prep_host(np.asarray(nbr))
    nc = _build_program(plan)
    _CACHE[key] = (nc, gaslab, gbslab, segslab)
    return _CACHE[key]


def kernel(x, W, gamma, beta, nbr):
    x = np.ascontiguousarray(np.asarray(x, np.float32))
    Wb = np.asarray(W, np.float32).astype(BF)
    gamma = np.asarray(gamma, np.float32).reshape(1, D)
    beta = np.asarray(beta, np.float32).reshape(1, D)
    nbr = np.asarray(nbr)
    nc, gaslab, gbslab, segslab = build(nbr)
    in_maps = []
    for c in range(C):
        xc = np.zeros((VP, D), BF)
        xc[:V] = x[c * V:(c + 1) * V].astype(BF)
        in_maps.append({
            "x_d": x,
            "xc_d": xc,
            "W_d": Wb,
            "gam_d": gamma,
            "bet_d": beta,
            "ga_d": gaslab[c],
            "gb_d": gbslab[c],
            "seg_d": segslab[c],
        })
    res = bass_utils.run_bass_kernel_spmd(nc, in_maps, core_ids=list(range(C)))
    return np.concatenate([res.results[c]["y_d"][:V] for c in range(C)], axis=0)
